# revision 1
# baseline (speedup 1.0000x reference)
"""Masked dot-product attention on 8 Trainium2 NeuronCores (Bass/Tile).

Problem: queries/keys/values [32, 1024, 128] f32, valid_lens [32] i32.
  out = softmax(mask(Q K^T / sqrt(128))) V        (key-padding prefix mask)

Strategy (batch-parallel, 4 batches per core, one SPMD program):
  * Host pre-transposes Q and K per batch to [D=128, 1024] (bf16) so the
    contraction dim D sits on SBUF partitions; no on-device transposes.
  * Scores are computed transposed: S^T[k, q] = (K^T chunk).T @ Q^T with k
    in chunks of 128 partitions, accumulating in f32 PSUM.
  * The prefix key mask is per-PARTITION in this layout, so it folds into
    the exp for free: ACT computes exp(S^T * 1/sqrt(D) + bias) with
    bias[k] in {0, -1e6}; masked rows become exactly 0. Probs are bf16.
  * out^T[v, q] += V_chunk-as-lhsT @ expS^T accumulates in PSUM across
    k chunks (V is loaded chunk-major, no transpose needed).
  * denominator[q]: expS^T chunks are reduced with a running-sum chain
    on DVE (bf16, 2x mode; one add per chunk), so the PE does only ONE
    two-matmul partition-reduction per batch into a [2, 512] PSUM bank
    (lhsT columns [1,0] / [0,1]), deferred past the epilogue so it never
    stalls the in-order PE queue at a batch boundary.
  * out^T and sums are DMA'd back in f32; the host divides and
    transposes while gathering (0.003% of the FLOPs).
  * All matmul operands are bf16 (1 cycle/row, FWL weight loads); PSUM
    accumulation stays f32, final output is f32. End-to-end rel err vs
    the f32 reference is ~3e-3 (tolerance 2e-2).

Startup/teardown engineering (the steady state is ACT-exp-bound at
~1.1us/chunk, so wins come from the edges):
  * Input DMAs are split across BOTH HWDGE rings: kt/qt stream on the
    Sync ring, mask/V/consts on the Scalar ring, interleaved per batch
    so vp[b] lands right after qt[b].
  * Every input tile is fully resident (per-slot tags, bufs=1), so no
    DMA issue ever waits on a buffer-reuse semaphore; those waits would
    head-of-line-block the issuing engine's queue.
  * PE + ACT warmup runs on a memset tile with NO DMA dependency: dummy
    matmuls fill the initial DMA window so the PE's HAM clock-gate
    reaches 8/8 before real work, and a dummy exp triggers the one-time
    ~1.3us ACT_TABLE_LOAD immediately after the framework preamble.
  * Epilogue PSUM->SBUF copies run on DVE only, emitted before the
    batch-final add so the accumulator bank frees early.
  * Only two tile pools (one SBUF, one PSUM): pool boundaries cost
    cross-engine barriers in the teardown.

Static masked-chunk skipping: batch b only needs ceil(valid_lens[b]/128)
key chunks; the rest contribute exactly 0. Batches are assigned to the 4
per-core slots by descending need (sorted, slot-major), so slot j's
compile-time chunk count is max over its 8 batches. The SPMD program is
specialized to that profile at kernel build time.

The chunk loop is software-pipelined: chunk c+1's score matmuls are
emitted before chunk c's AV/sums matmuls so the PE produces the next
exp's input first and ACT never starves.
"""

import math

import ml_dtypes
import numpy as np

import concourse.bacc as bacc
import concourse.bass as bass
import concourse.mybir as mybir
import concourse.tile as tile
from concourse.bass_utils import run_bass_kernel_spmd

B, Q, K, D = 32, 1024, 1024, 128
N_CORES = 8
BPC = B // N_CORES  # batches per core
PART = 128          # partition size / key chunk size
NCHUNK = K // PART
MASK_BIAS = -1.0e6
INV_SQRT_D = 1.0 / math.sqrt(D)
F32 = mybir.dt.float32
BF16 = mybir.dt.bfloat16
NP_BF16 = ml_dtypes.bfloat16
N_WARM_MM = 8       # dummy PE matmuls (512 cols each): bridge the whole
                    # initial DMA window so the HAM activity stays unbroken
P_BUFS = 12         # probs-tile ring size

_NC_CACHE: dict = {}


def build_nc(profile: tuple) -> bass.Bass:
    """Build the SPMD Bass program for a per-slot chunk-count profile."""
    nc = bacc.Bacc()
    qt = nc.declare_dram_parameter("qt", [BPC, PART, Q], BF16, isOutput=False)
    kt = nc.declare_dram_parameter("kt", [BPC, PART, K], BF16, isOutput=False)
    vp = nc.declare_dram_parameter("vp", [BPC, PART, K], BF16, isOutput=False)
    mb = nc.declare_dram_parameter("mb", [PART, BPC * NCHUNK], F32, isOutput=False)
    cst = nc.declare_dram_parameter("cst", [PART, 4], BF16, isOutput=False)
    out = nc.declare_dram_parameter("out", [BPC, PART, Q], BF16, isOutput=True)
    sums_out = nc.declare_dram_parameter("sums", [BPC, 2, 512], F32, isOutput=True)

    with tile.TileContext(nc) as tc:
        with (
            tc.tile_pool(name="sb", bufs=1) as sb,
            tc.tile_pool(name="ps", bufs=1, space="PSUM") as ps,
        ):
            # Warmup with no DMA dependency: memset a tile, then hammer the
            # PE with dummy matmuls (HAM warm) and ACT with a dummy exp
            # (one-time exp table load) while the first inputs stream in.
            # The warm/filler PSUM bank is never read by any engine.
            warm_sb = sb.tile([PART, 512], BF16, tag="warm")
            nc.vector.memset(warm_sb, 1.0)
            warm_ps = ps.tile([PART, 512], F32, tag="fill", bufs=1)
            for _ in range(N_WARM_MM):
                nc.tensor.matmul(
                    warm_ps, warm_sb[:, 0:PART], warm_sb, start=True, stop=True
                )

            def filler_mm(i):
                # Dep-free dummy matmul slotted where the in-order PE queue
                # would otherwise micro-idle waiting on the current exp.
                # Keeps the PE's HAM activity window busy so the clock-gate
                # stays at 8/8; an idle-throttled PE (427ns vs 216ns per
                # matmul) cannot keep ahead of ACT and starves the exps.
                f_ps = ps.tile([PART, PART], F32, tag="fill", bufs=1,
                               name=f"fill_{i}")
                nc.tensor.matmul(
                    f_ps, warm_sb[:, 0:PART], warm_sb[:, 0:PART],
                    start=True, stop=True,
                )
            # Consts: the mb issue goes first on the ACT queue (its data
            # gates the first real exp); the dummy exp (which pulls in the
            # one-time ACT_TABLE_LOAD) follows it.
            mb_sb = sb.tile([PART, BPC * NCHUNK], F32, tag="mb")
            nc.scalar.dma_start(out=mb_sb, in_=mb[:, :])
            warm_act = sb.tile([PART, 1], F32, tag="warm_act")
            nc.scalar.activation(
                warm_act,
                warm_sb[:, 0:1],
                mybir.ActivationFunctionType.Exp,
                scale=0.0,
            )

            # Input streams. kt0/qt0 lead the Sync ring (the ring's FIFO
            # self-serializes the later batches' transfers behind them, so
            # the startup-critical loads get nearly full SDMA bandwidth);
            # vp[b] follows qt[b] so V lands before its batch's AV matmuls.
            # The Scalar ring carries only the tiny mb/vp0/cst loads.
            ins_sb = []
            for b in range(BPC):
                cap = profile[b]
                kcols = cap * PART
                qt_sb = sb.tile([PART, Q], BF16, tag=f"qt{b}", name=f"qt{b}")
                kt_sb = sb.tile([PART, kcols], BF16, tag=f"kt{b}", name=f"kt{b}")
                vp_sb = sb.tile([PART, kcols], BF16, tag=f"vp{b}", name=f"vp{b}")
                ins_sb.append((qt_sb, kt_sb, vp_sb))
                nc.sync.dma_start(out=kt_sb, in_=kt[b][:, :kcols])
                if b == 0:
                    # qt0 in halves: the first score matmul only needs the
                    # first half, so it starts one half-transfer earlier.
                    nc.sync.dma_start(out=qt_sb[:, 0:512], in_=qt[b][:, 0:512])
                    nc.sync.dma_start(
                        out=qt_sb[:, 512:1024], in_=qt[b][:, 512:1024]
                    )
                    nc.scalar.dma_start(out=vp_sb, in_=vp[b][:, :kcols])
                else:
                    nc.sync.dma_start(out=qt_sb, in_=qt[b])
                    nc.sync.dma_start(out=vp_sb, in_=vp[b][:, :kcols])
                    if b == 1:
                        # cst is first needed by the first sums flush
                        # (several chunks in); keep its issue off the ACT
                        # queue so the first exp isn't queued behind it.
                        cst_sb = sb.tile([PART, 4], BF16, tag="cst")
                        nc.sync.dma_start(out=cst_sb, in_=cst[:, :])

            # Flat chunk stream across batches with 2-deep score lookahead:
            # the in-order PE queue must see the next chunks' score matmuls
            # BEFORE a batch-boundary AV matmul that may stall on the PSUM
            # accumulator release.
            stream = [(b, c) for b in range(BPC) for c in range(profile[b])]

            def s_mms(b, c):
                qt_sb, kt_sb, _ = ins_sb[b]
                s_ps = ps.tile([PART, Q], F32, tag="s", bufs=2, name=f"s_b{b}c{c}")
                kw = kt_sb[:, c * PART:(c + 1) * PART]
                for h in range(2):
                    nc.tensor.matmul(
                        s_ps[:, h * 512:(h + 1) * 512],
                        kw,
                        qt_sb[:, h * 512:(h + 1) * 512],
                        start=True,
                        stop=True,
                    )
                return s_ps

            def p_tile(nm):
                return sb.tile([PART, Q], BF16, tag="p", bufs=P_BUFS, name=nm)

            def sums_mms(sums_ps, rhs_t, st, sp):
                # Rows [sum of h0 cols; sum of h1 cols] into one PSUM bank:
                # lhsT columns are [1,0] (cst cols 0:2) and [0,1] (cols 2:4).
                nc.tensor.matmul(
                    sums_ps[0:2, 0:512],
                    cst_sb[:, 0:2],
                    rhs_t[:, 0:512],
                    start=st,
                    stop=False,
                )
                nc.tensor.matmul(
                    sums_ps[0:2, 0:512],
                    cst_sb[:, 2:4],
                    rhs_t[:, 512:1024],
                    start=False,
                    stop=sp,
                )

            def sums_epilogue(b, sums_ps):
                sums_sb = sb.tile(
                    [2, 512], F32, tag="sums_sb", bufs=2, name=f"sums_sb{b}"
                )
                if b == BPC - 1:
                    # Final batch: ACT is idle after the last exp, while DVE
                    # still has both output casts queued and Sync both output
                    # issues. Copy + issue on the Scalar side instead so the
                    # kernel's last DMA lands ~0.35us earlier.
                    nc.scalar.copy(sums_sb, sums_ps)
                    nc.scalar.dma_start(out=sums_out[b], in_=sums_sb)
                else:
                    nc.vector.tensor_copy(sums_sb, sums_ps)
                    nc.sync.dma_start(out=sums_out[b], in_=sums_sb)

            # pend entries: (due_i, sums_ps, rhs, st, sp, b_if_final_group)
            pend = []

            def flush_pend(i):
                keep = []
                for e in pend:
                    if e[0] <= i:
                        _, ps_t, rhs_t, st, sp, eb = e
                        sums_mms(ps_t, rhs_t, st, sp)
                        if eb is not None:
                            sums_epilogue(eb, ps_t)
                    else:
                        keep.append(e)
                pend[:] = keep

            s_tiles = {}
            for j in range(min(2, len(stream))):
                s_tiles[stream[j]] = s_mms(*stream[j])
            acc = {}
            run = {}  # per-batch running prob-sum tile
            for i, (b, c) in enumerate(stream):
                cap = profile[b]
                if c == 0:
                    # Two independent half-accumulators (one PSUM bank each):
                    # each half frees as soon as its own epilogue copy is
                    # done, so the next batch's first AV h0 starts ~0.6us
                    # earlier than with a single 2-bank accumulator.
                    out_ps = (
                        ps.tile([PART, 512], F32, tag="outA", bufs=1,
                                name=f"outA_b{b}"),
                        ps.tile([PART, 512], F32, tag="outB", bufs=1,
                                name=f"outB_b{b}"),
                    )
                    sums_ps = ps.tile(
                        [2, 512], F32, tag="sums", bufs=1, name=f"sums_b{b}"
                    )
                    acc[b] = (out_ps, sums_ps)
                out_ps, sums_ps = acc[b]
                p_sb = p_tile(f"p_{i}")
                nc.scalar.activation(
                    p_sb,
                    s_tiles.pop((b, c)),
                    mybir.ActivationFunctionType.Exp,
                    bias=mb_sb[:, b * NCHUNK + c:b * NCHUNK + c + 1],
                    scale=INV_SQRT_D,
                )
                if i + 2 < len(stream):
                    s_tiles[stream[i + 2]] = s_mms(*stream[i + 2])
                flush_pend(i)
                vw = ins_sb[b][2][:, c * PART:(c + 1) * PART]
                first, last = c == 0, c == cap - 1
                for h in range(2):
                    nc.tensor.matmul(
                        out_ps[h],
                        vw,
                        p_sb[:, h * 512:(h + 1) * 512],
                        start=first,
                        stop=last,
                    )
                if last:
                    # Epilogue out-copies first: the accumulator bank frees
                    # before the batch-final DVE add runs. For the stream's
                    # final batch the h1 DMA issues on the (now exp-free)
                    # Scalar ring so it overlaps h0's issue on Sync.
                    outn = sb.tile([PART, Q], BF16, tag="outn", bufs=3, name=f"outn{b}")
                    nc.vector.tensor_copy(outn[:, 0:512], out_ps[0])
                    nc.sync.dma_start(out=out[b][:, 0:512], in_=outn[:, 0:512])
                    nc.vector.tensor_copy(outn[:, 512:1024], out_ps[1])
                    nc.sync.dma_start(
                        out=out[b][:, 512:1024], in_=outn[:, 512:1024]
                    )
                # Running-sum chain on DVE: one add per chunk, so only one
                # add remains at the batch boundary (a deep fold chain there
                # would delay the epilogue copies and the PSUM release).
                if b not in run:
                    cur = p_sb
                else:
                    cur = p_tile(f"run_{i}")
                    nc.vector.tensor_add(cur, run.pop(b), p_sb)
                if not last:
                    run[b] = cur
                else:
                    pend.append((i + 1, sums_ps, cur, True, True, b))

            flush_pend(len(stream) + 3)

    nc.compile()
    return nc


def plan(valid_lens: np.ndarray):
    """Assign batches to (core, slot) and derive the chunk-count profile.

    Sorting by descending need and slicing slot-major minimizes the sum of
    per-slot maxima, which is the per-core static work.
    """
    need = np.minimum((valid_lens.astype(np.int64) + PART - 1) // PART, NCHUNK)
    need = np.maximum(need, 1)
    order = np.argsort(-need, kind="stable")
    perm = order.reshape(BPC, N_CORES)  # perm[slot, core] = batch index
    # Process the smallest slot first: its input DMAs are the ones compute
    # must wait for at startup; the bigger slots' loads overlap compute.
    rot = np.argsort([int(need[perm[s]].max()) for s in range(BPC)], kind="stable")
    rot = np.concatenate([rot[:1], rot[1:][::-1]])  # smallest, then descending
    perm = perm[rot]
    profile = tuple(int(need[perm[s]].max()) for s in range(BPC))
    return perm, profile


def host_prep(q, k, v, lens):
    """Shard + lay out inputs for the 8 cores. Returns (perm, profile, in_maps)."""
    perm, profile = plan(lens)

    # Vectorized host layout prep: obi[core, slot] = batch index.
    obi = perm.T  # [N_CORES, BPC]
    qt_all = np.ascontiguousarray(
        q[obi].transpose(0, 1, 3, 2).astype(NP_BF16)
    )  # [8,4,128,1024]
    kt_all = np.ascontiguousarray(k[obi].transpose(0, 1, 3, 2).astype(NP_BF16))
    # v chunk-major: vp[p, c*128 + d] = v[c*128 + p, d]
    vp_all = np.ascontiguousarray(
        v[obi]
        .reshape(N_CORES, BPC, NCHUNK, PART, D)
        .transpose(0, 1, 3, 2, 4)
        .reshape(N_CORES, BPC, PART, K)
        .astype(NP_BF16)
    )
    # bias[p, slot*8 + c] = 0 if (c*128+p) < L else -1e6
    valid = np.arange(K)[None, None, :] < lens[obi][:, :, None]  # [8,4,1024]
    mb_all = np.where(
        valid.reshape(N_CORES, BPC, NCHUNK, PART).transpose(0, 2, 3, 1), 0.0, MASK_BIAS
    ).astype(np.float32)  # [8, NCHUNK, PART, BPC] -> need [8, PART, BPC*NCHUNK]
    mb_all = np.ascontiguousarray(
        mb_all.transpose(0, 2, 3, 1).reshape(N_CORES, PART, BPC * NCHUNK)
    )
    # cst columns [1,0] and [0,1]: the sums-matmul weight pairs.
    cstv = np.zeros((PART, 4), NP_BF16)
    cstv[:, 0] = 1
    cstv[:, 3] = 1

    in_maps = [
        {
            "qt": qt_all[core],
            "kt": kt_all[core],
            "vp": vp_all[core],
            "mb": mb_all[core],
            "cst": cstv,
        }
        for core in range(N_CORES)
    ]
    return perm, profile, in_maps


def kernel(queries, keys, values, valid_lens):
    q = np.ascontiguousarray(np.asarray(queries, dtype=np.float32))
    k = np.ascontiguousarray(np.asarray(keys, dtype=np.float32))
    v = np.ascontiguousarray(np.asarray(values, dtype=np.float32))
    lens = np.asarray(valid_lens).astype(np.int64).reshape(B)

    perm, profile, in_maps = host_prep(q, k, v, lens)

    if profile not in _NC_CACHE:
        _NC_CACHE[profile] = build_nc(profile)
    nc = _NC_CACHE[profile]

    res = run_bass_kernel_spmd(nc, in_maps, list(range(N_CORES)))

    out = np.empty((B, Q, D), np.float32)
    for core in range(N_CORES):
        core_out = res.results[core]["out"]    # [BPC, 128(v), 1024(q)]
        core_sums = res.results[core]["sums"]  # [BPC, 2, 512] -> flat [1024(q)]
        for slot in range(BPC):
            bidx = int(perm[slot, core])
            sums_flat = core_sums[slot].reshape(Q)
            out[bidx] = (
                core_out[slot].astype(np.float32) / sums_flat[None, :]
            ).T
    return out



# revision 4
# speedup vs baseline: 1.0422x; 1.0422x over previous
"""Masked dot-product attention on 8 Trainium2 NeuronCores (Bass/Tile).

Problem: queries/keys/values [32, 1024, 128] f32, valid_lens [32] i32.
  out = softmax(mask(Q K^T / sqrt(128))) V        (key-padding prefix mask)

Strategy (fragment-parallel flash decomposition, one SPMD program):
  * The unit of work is a key CHUNK (128 keys) of one batch. Since the
    kernel's softmax uses no running-max (scores ~ N(0,1), exp is safe),
    a batch's chunks can be split across cores: each fragment produces a
    partial out^T = V_r^T @ exp(S_r^T) and partial denominator sums; the
    host adds partials across fragments and divides. This drops per-core
    work from the sum of per-slot whole-batch maxima (20 chunks) to the
    exact floor ceil(total_chunks/8) (17 for the reference lens).
  * plan() finds a slot profile (p_1..p_S) and an exact cutting of the
    32 batches into fragments filling all 8*S slot instances (DP when
    zero slack, greedy otherwise, whole-batch fallback).
  * Per (core, slot): one fused input bundle [qt | kt | vp] (bf16, host
    pre-transposed) loaded with ONE dma_start on the Sync HWDGE ring in
    ascending-slot order; slot 0's bundle also carries the exp bias table
    mb (0 / -1e6, bf16 is exact enough: exp(x-999424)==0) and the
    sums-matmul constant columns. This cuts ~14 input issues (~0.6us of
    queue time each) to ~6 and keeps the ACT queue free of DMA issues so
    the first real exp can run right after ACT_TABLE_LOAD + dummy exp.
  * Scores are computed transposed: S^T[k, q] = kt_chunk.T @ qt with k on
    partitions; the prefix mask folds into the exp bias for free; probs
    are bf16; out^T accumulates over the fragment's chunks in two
    half-PSUM accumulators that free independently; the denominator uses
    a DVE running-sum chain + one deferred 2-matmul partition-reduction
    per fragment into a [2, 512] PSUM bank (see baseline notes below).
  * PE + ACT warmup: dummy matmuls on a memset tile (no DMA dependency)
    bridge the initial DMA window so the HAM clock-gate reaches 8/8, and
    a dummy exp pulls the one-time ACT_TABLE_LOAD forward.
  * The chunk loop is software-pipelined with 2-deep score lookahead so
    ACT never starves; epilogue PSUM->SBUF copies run on DVE (Scalar for
    the final slot, whose exps are done) before the fragment-final add.

Host gather: out[b] = (sum_frag out_frag / sum_frag sums_frag)^T, f32.
"""

import math

import ml_dtypes
import numpy as np

import concourse.bacc as bacc
import concourse.bass as bass
import concourse.mybir as mybir
import concourse.tile as tile
from concourse.bass_utils import run_bass_kernel_spmd

B, Q, K, D = 32, 1024, 1024, 128
N_CORES = 8
PART = 128          # partition size / key chunk size
NCHUNK = K // PART
MASK_BIAS = -1.0e6
INV_SQRT_D = 1.0 / math.sqrt(D)
F32 = mybir.dt.float32
BF16 = mybir.dt.bfloat16
NP_BF16 = ml_dtypes.bfloat16
N_WARM_MM = 4       # dummy PE matmuls (512 cols each): bridge the initial
                    # DMA window so the HAM activity stays unbroken
P_BUFS = 12         # probs-tile ring size

_NC_CACHE: dict = {}


def build_nc(profile: tuple) -> bass.Bass:
    """Build the SPMD Bass program for a slot chunk-count profile."""
    S = len(profile)
    nc = bacc.Bacc()
    # Per-slot fused input bundle: [qt (Q) | kt (p*128) | vp (p*128)];
    # slot 0 additionally carries [mb (S*8) | cst (4)].
    ins = []
    for s, p in enumerate(profile):
        cols = Q + 2 * p * PART + (S * NCHUNK + 4 if s == 0 else 0)
        ins.append(nc.declare_dram_parameter(f"in{s}", [PART, cols], BF16,
                                             isOutput=False))
    out = nc.declare_dram_parameter("out", [S, PART, Q], BF16, isOutput=True)
    sums_out = nc.declare_dram_parameter("sums", [S, 2, 512], F32, isOutput=True)

    with tile.TileContext(nc) as tc:
        with (
            tc.tile_pool(name="sb", bufs=1) as sb,
            tc.tile_pool(name="ps", bufs=1, space="PSUM") as ps,
        ):
            # Warmup with no DMA dependency: memset a tile, then dummy
            # matmuls (HAM warm) + a dummy exp (one-time exp table load)
            # while the first inputs stream in.
            warm_sb = sb.tile([PART, 512], BF16, tag="warm")
            nc.vector.memset(warm_sb, 1.0)
            warm_ps = ps.tile([PART, 512], F32, tag="fill", bufs=1)
            for _ in range(N_WARM_MM):
                nc.tensor.matmul(
                    warm_ps, warm_sb[:, 0:PART], warm_sb, start=True, stop=True
                )
            warm_act = sb.tile([PART, 1], F32, tag="warm_act")
            nc.scalar.activation(
                warm_act,
                warm_sb[:, 0:1],
                mybir.ActivationFunctionType.Exp,
                scale=0.0,
            )

            # Input bundles, all on the Sync HWDGE ring, ascending slot
            # order (slot 0 is smallest; later slots prefetch behind it).
            ins_sb = []
            for s, p in enumerate(profile):
                t = sb.tile(list(ins[s].shape), BF16, tag=f"in{s}", name=f"in{s}")
                ins_sb.append(t)
            nc.sync.dma_start(out=ins_sb[0], in_=ins[0][:, :])
            for s in range(1, S):
                nc.sync.dma_start(out=ins_sb[s], in_=ins[s][:, :])
            mb_off = Q + 2 * profile[0] * PART
            mb_sb = ins_sb[0][:, mb_off:mb_off + S * NCHUNK]
            cst_sb = ins_sb[0][:, mb_off + S * NCHUNK:mb_off + S * NCHUNK + 4]

            def qt_w(s):
                return ins_sb[s][:, 0:Q]

            def kt_w(s, c):
                return ins_sb[s][:, Q + c * PART:Q + (c + 1) * PART]

            def vp_w(s, c):
                off = Q + profile[s] * PART
                return ins_sb[s][:, off + c * PART:off + (c + 1) * PART]

            # Flat chunk stream across slots with 2-deep score lookahead:
            # the in-order PE queue must see the next chunks' score
            # matmuls BEFORE a slot-boundary AV matmul that may stall on
            # the PSUM accumulator release.
            stream = [(s, c) for s in range(S) for c in range(profile[s])]

            def s_mms(s, c):
                s_ps = ps.tile([PART, Q], F32, tag="s", bufs=2,
                               name=f"s_s{s}c{c}")
                kw = kt_w(s, c)
                qt = qt_w(s)
                for h in range(2):
                    nc.tensor.matmul(
                        s_ps[:, h * 512:(h + 1) * 512],
                        kw,
                        qt[:, h * 512:(h + 1) * 512],
                        start=True,
                        stop=True,
                    )
                return s_ps

            def p_tile(nm):
                return sb.tile([PART, Q], BF16, tag="p", bufs=P_BUFS, name=nm)

            def sums_mms(sums_ps, rhs_t, st, sp):
                # Rows [sum of h0 cols; sum of h1 cols] into one PSUM
                # bank: lhsT columns are [1,0] and [0,1] of cst.
                nc.tensor.matmul(
                    sums_ps[0:2, 0:512],
                    cst_sb[:, 0:2],
                    rhs_t[:, 0:512],
                    start=st,
                    stop=False,
                )
                nc.tensor.matmul(
                    sums_ps[0:2, 0:512],
                    cst_sb[:, 2:4],
                    rhs_t[:, 512:1024],
                    start=False,
                    stop=sp,
                )

            def sums_epilogue(s, sums_ps):
                sums_sb = sb.tile(
                    [2, 512], F32, tag="sums_sb", bufs=2, name=f"sums_sb{s}"
                )
                if s == S - 1:
                    # Final slot: ACT is idle after the last exp while DVE
                    # still has both output casts queued and Sync both
                    # output issues; copy + issue on the Scalar side.
                    nc.scalar.copy(sums_sb, sums_ps)
                    nc.scalar.dma_start(out=sums_out[s], in_=sums_sb)
                else:
                    nc.vector.tensor_copy(sums_sb, sums_ps)
                    nc.sync.dma_start(out=sums_out[s], in_=sums_sb)

            # pend entries: (due_i, sums_ps, rhs, st, sp, s_if_final)
            pend = []

            def flush_pend(i):
                keep = []
                for e in pend:
                    if e[0] <= i:
                        _, ps_t, rhs_t, st, sp, es = e
                        sums_mms(ps_t, rhs_t, st, sp)
                        if es is not None:
                            sums_epilogue(es, ps_t)
                    else:
                        keep.append(e)
                pend[:] = keep

            s_tiles = {}
            for j in range(min(2, len(stream))):
                s_tiles[stream[j]] = s_mms(*stream[j])
            acc = {}
            run = {}  # per-slot running prob-sum tile
            for i, (s, c) in enumerate(stream):
                cap = profile[s]
                if c == 0:
                    # Two independent half-accumulators (one PSUM bank
                    # each): each half frees as soon as its own epilogue
                    # copy is done.
                    out_ps = (
                        ps.tile([PART, 512], F32, tag="outA", bufs=1,
                                name=f"outA_s{s}"),
                        ps.tile([PART, 512], F32, tag="outB", bufs=1,
                                name=f"outB_s{s}"),
                    )
                    sums_ps = ps.tile(
                        [2, 512], F32, tag="sums", bufs=1, name=f"sums_s{s}"
                    )
                    acc[s] = (out_ps, sums_ps)
                out_ps, sums_ps = acc[s]
                p_sb = p_tile(f"p_{i}")
                nc.scalar.activation(
                    p_sb,
                    s_tiles.pop((s, c)),
                    mybir.ActivationFunctionType.Exp,
                    bias=mb_sb[:, s * NCHUNK + c:s * NCHUNK + c + 1],
                    scale=INV_SQRT_D,
                )
                if i + 2 < len(stream):
                    s_tiles[stream[i + 2]] = s_mms(*stream[i + 2])
                flush_pend(i)
                vw = vp_w(s, c)
                first, last = c == 0, c == cap - 1
                for h in range(2):
                    nc.tensor.matmul(
                        out_ps[h],
                        vw,
                        p_sb[:, h * 512:(h + 1) * 512],
                        start=first,
                        stop=last,
                    )
                if last:
                    # Epilogue out-copies first: the accumulator bank
                    # frees before the fragment-final DVE add runs.
                    outn = sb.tile([PART, Q], BF16, tag="outn", bufs=3,
                                   name=f"outn{s}")
                    nc.vector.tensor_copy(outn[:, 0:512], out_ps[0])
                    nc.sync.dma_start(out=out[s][:, 0:512], in_=outn[:, 0:512])
                    nc.vector.tensor_copy(outn[:, 512:1024], out_ps[1])
                    nc.sync.dma_start(
                        out=out[s][:, 512:1024], in_=outn[:, 512:1024]
                    )
                # Running-sum chain on DVE: one add per chunk, so only
                # one add remains at the fragment boundary.
                if s not in run:
                    cur = p_sb
                else:
                    cur = p_tile(f"run_{i}")
                    nc.vector.tensor_add(cur, run.pop(s), p_sb)
                if not last:
                    run[s] = cur
                else:
                    pend.append((i + 1, sums_ps, cur, True, True, s))

            flush_pend(len(stream) + 3)

    nc.compile()
    return nc


def _profiles(S, T, maxp):
    """Descending profiles of length S summing to T, parts in [1, maxp]."""
    out = []

    def rec(rem_slots, rem_sum, hi, cur):
        if rem_slots == 0:
            if rem_sum == 0:
                out.append(tuple(cur))
            return
        lo = max(1, rem_sum - (rem_slots - 1) * hi)
        for p in range(min(hi, rem_sum - (rem_slots - 1)), lo - 1, -1):
            cur.append(p)
            rec(rem_slots - 1, rem_sum - p, p, cur)
            cur.pop()

    rec(S, T, maxp, [])
    return out


def _exact_assign(needs, profile):
    """Zero-slack exact cover: cut batches (needs, desc order of (need,
    batch)) into parts exactly matching 8 copies of each profile entry.
    Returns per-batch composition counts over distinct sizes, or None."""
    sizes = sorted(set(profile), reverse=True)
    cap = tuple(8 * profile.count(sz) for sz in sizes)

    comp_cache = {}

    def comps(n):
        if n in comp_cache:
            return comp_cache[n]
        res = []

        def rec(i, rem, cur):
            if rem == 0:
                res.append(tuple(cur) + (0,) * (len(sizes) - len(cur)))
                return
            if i == len(sizes):
                return
            for k in range(rem // sizes[i], -1, -1):
                cur.append(k)
                rec(i + 1, rem - k * sizes[i], cur)
                cur.pop()

        rec(0, n, [])
        comp_cache[n] = res
        return res

    order = sorted(range(len(needs)), key=lambda b: -needs[b])
    fail = set()

    def solve(idx, rem):
        if idx == len(order):
            return [] if all(r == 0 for r in rem) else None
        key = (idx, rem)
        if key in fail:
            return None
        for comp in comps(needs[order[idx]]):
            if all(ci <= ri for ci, ri in zip(comp, rem)):
                tail = solve(idx + 1,
                             tuple(ri - ci for ri, ci in zip(rem, comp)))
                if tail is not None:
                    return [comp] + tail
        fail.add(key)
        return None

    sol = solve(0, cap)
    if sol is None:
        return None
    return sizes, {order[i]: sol[i] for i in range(len(order))}


def _greedy_assign(needs, profile):
    """Slack-tolerant greedy: largest remaining need to largest instance.
    Returns list of (instance_slot_index, batch, frag_len) or None."""
    inst = sorted(
        ((p, s, k) for s, p in enumerate(profile) for k in range(8)),
        reverse=True,
    )
    rem = {b: n for b, n in enumerate(needs)}
    frags = []
    for p, s, k in inst:
        if not rem:
            break
        b = max(rem, key=lambda x: rem[x])
        take = min(p, rem[b])
        frags.append((s, b, take))
        rem[b] -= take
        if rem[b] == 0:
            del rem[b]
    if rem:
        return None
    return frags


def plan(valid_lens: np.ndarray):
    """Choose slot profile + cut batches into fragments.

    Returns (profile_asc, frags) where frags[core][slot] = (b, c0, f)
    (f may be 0 for an empty padded instance).
    """
    need = np.maximum(
        np.minimum((valid_lens.astype(np.int64) + PART - 1) // PART, NCHUNK), 1
    )
    needs = need.tolist()
    total = int(need.sum())
    T0 = -(-total // N_CORES)

    frag_list = None   # list of (slot, batch, frag_len)
    profile = None
    if T0 * N_CORES == total:
        for S in (4, 5, 6):
            # pick the most balanced exact profile (lexicographically
            # smallest in descending representation)
            hits = [(prof, _exact_assign(needs, prof))
                    for prof in _profiles(S, T0, NCHUNK)]
            hits = [(prof, r) for prof, r in hits if r is not None]
            for prof, r in sorted(hits):
                if True:
                    sizes, comp_by_batch = r
                    # expand: fragments per size -> instances
                    by_size = {sz: [] for sz in sizes}
                    for b in sorted(comp_by_batch, key=lambda b: -needs[b]):
                        c0 = 0
                        for sz, cnt in zip(sizes, comp_by_batch[b]):
                            for _ in range(cnt):
                                by_size[sz].append((b, c0, sz))
                                c0 += sz
                    frag_list = []
                    for s, p in enumerate(prof):
                        for (b, c0, f) in by_size[p][:8]:
                            frag_list.append((s, b, c0, f))
                        by_size[p] = by_size[p][8:]
                    profile = prof
                    break
            if frag_list is not None:
                break
    if frag_list is None:
        for T in range(T0, T0 + 4):
            for S in (4, 5, 6):
                done = False
                for prof in _profiles(S, T, NCHUNK):
                    g = _greedy_assign(needs, prof)
                    if g is not None:
                        cursor = {}
                        frag_list = []
                        for (s, b, f) in g:
                            c0 = cursor.get(b, 0)
                            frag_list.append((s, b, c0, f))
                            cursor[b] = c0 + f
                        profile = prof
                        done = True
                        break
                if done:
                    break
            if frag_list is not None:
                break
    # order slots ascending by chunk count (first slot smallest: startup)
    S = len(profile)
    order = sorted(range(S), key=lambda s: profile[s])
    remap = {old: new for new, old in enumerate(order)}
    profile_asc = tuple(profile[s] for s in order)
    per_slot = {s: [] for s in range(S)}
    for (s, b, c0, f) in frag_list:
        per_slot[remap[s]].append((b, c0, f))
    frags = [[None] * S for _ in range(N_CORES)]
    for s in range(S):
        lst = per_slot[s]
        while len(lst) < N_CORES:
            lst.append((0, 0, 0))   # empty padded instance
        for core in range(N_CORES):
            frags[core][s] = lst[core]
    return profile_asc, frags


def host_prep(q, k, v, lens):
    """Shard + lay out inputs for the 8 cores."""
    profile, frags = plan(lens)
    S = len(profile)
    kidx = np.arange(K)

    in_maps = []
    for core in range(N_CORES):
        m = {}
        for s, p in enumerate(profile):
            b, c0, f = frags[core][s]
            extra = S * NCHUNK + 4 if s == 0 else 0
            cols = Q + 2 * p * PART + extra
            buf = np.zeros((PART, cols), NP_BF16)
            buf[:, 0:Q] = q[b].T
            lo, hi = c0 * PART, (c0 + f) * PART
            buf[:, Q:Q + f * PART] = k[b][lo:hi].T
            # vp chunk-major: block j = v[(c0+j)*128 : ..., :] (k on part)
            if f:
                buf[:, Q + p * PART:Q + p * PART + f * PART] = (
                    v[b][lo:hi]
                    .reshape(f, PART, D)
                    .transpose(1, 0, 2)
                    .reshape(PART, f * D)
                )
            m[f"in{s}"] = buf
        # mb + cst live in slot 0's bundle
        buf0 = m["in0"]
        off = Q + 2 * profile[0] * PART
        mb = np.full((PART, S * NCHUNK), MASK_BIAS, np.float32)
        for s, p in enumerate(profile):
            b, c0, f = frags[core][s]
            L = int(lens[b])
            for j in range(f):
                valid = (c0 + j) * PART + np.arange(PART) < L
                mb[:, s * NCHUNK + j] = np.where(valid, 0.0, MASK_BIAS)
        buf0[:, off:off + S * NCHUNK] = mb.astype(NP_BF16)
        cst = np.zeros((PART, 4), NP_BF16)
        cst[:, 0] = 1
        cst[:, 3] = 1
        buf0[:, off + S * NCHUNK:off + S * NCHUNK + 4] = cst
        in_maps.append(m)
    return profile, frags, in_maps


def gather(results, profile, frags):
    """Sum partial (out, sums) across fragments; divide + transpose."""
    S = len(profile)
    out_acc = np.zeros((B, PART, Q), np.float32)
    sums_acc = np.zeros((B, Q), np.float32)
    for core in range(N_CORES):
        core_out = np.asarray(results[core]["out"], np.float32)
        core_sums = np.asarray(results[core]["sums"], np.float32)
        for s in range(S):
            b, c0, f = frags[core][s]
            if f == 0:
                continue
            out_acc[b] += core_out[s]
            sums_acc[b] += core_sums[s].reshape(Q)
    out = np.empty((B, Q, D), np.float32)
    for b in range(B):
        out[b] = (out_acc[b] / sums_acc[b][None, :]).T
    return out


def kernel(queries, keys, values, valid_lens):
    q = np.ascontiguousarray(np.asarray(queries, dtype=np.float32))
    k = np.ascontiguousarray(np.asarray(keys, dtype=np.float32))
    v = np.ascontiguousarray(np.asarray(values, dtype=np.float32))
    lens = np.asarray(valid_lens).astype(np.int64).reshape(B)

    profile, frags, in_maps = host_prep(q, k, v, lens)

    if profile not in _NC_CACHE:
        _NC_CACHE[profile] = build_nc(profile)
    nc = _NC_CACHE[profile]

    res = run_bass_kernel_spmd(nc, in_maps, list(range(N_CORES)))
    return gather(res.results, profile, frags)
